# revision 56
# baseline (speedup 1.0000x reference)
"""Trainium2 Bass kernel for nn_DecCLSTMBlock (3x stacked ConvLSTM2D + BN + 2x2 upsample).

Sharding: 8 cores = 2 batch groups x 4 H-shards.
  - L1 (16x16, Cout=128) and L2 (32x32, Cout=64) computed fully per core
    (replicated within the batch group).
  - L3 (64x64, Cout=32) sharded 4 ways over H with a shrinking-halo
    overcompute domain of 34 rows -> no inter-core communication.
  - L2 and L3 are interleaved per timestep: L3's elementwise-heavy chain
    overlaps L2's matmul-heavy stream. y2 stays in SBUF (no DRAM staging).
  - Final BN + upsample of layer 3 done host-side in numpy.
All conv weights are zero-padded to K=128 partitions so every LDWEIGHTS has
identical full-array geometry (lets the PE pull weight loads ahead of
in-flight matmuls). LSTM cell elementwise uses scalar_tensor_tensor fusion:
(U min 1) * v does hard_sigmoid clamp + gate multiply in one op.
Matmuls run in bf16 with fp32 PSUM accumulation (cell state c stays fp32).
Assumes bias vectors b1..b3 are zero (they are, per the problem spec).
"""
import os
import sys
import types

sys.path.insert(0, "/opt/trn_rl_repo")

import numpy as np

import concourse.bass as bass
import concourse.tile as tile
from concourse import bacc, mybir
from concourse.bass_utils import run_bass_kernel_spmd

F32 = mybir.dt.float32
BF16 = mybir.dt.bfloat16
I32 = mybir.dt.int32
AF = mybir.ActivationFunctionType
OP = mybir.AluOpType

B, T = 2, 8
EPS = 1e-3

_PROG = None


def _install_trace_hook():
    try:
        import antenv
        if 'antenv.axon_hooks' not in sys.modules:
            mod = types.ModuleType('antenv.axon_hooks')
            mod._hook = None
            def _set(h):
                mod._hook = h
            def _get():
                return mod._hook
            mod.set_axon_ntff_profile_hook = _set
            mod.get_axon_ntff_profile_hook = _get
            sys.modules['antenv.axon_hooks'] = mod
            antenv.axon_hooks = mod
            from trn_agent_boot.trn_boot import _ntff_profile_via_ctypes
            mod.set_axon_ntff_profile_hook(
                _ntff_profile_via_ctypes('/opt/axon/libaxon_pjrt.so'))
        import concourse.bass_utils as bu
        bu.upload_artifacts = lambda tmpdir: "local://" + tmpdir
        return True
    except Exception:
        return False


def build_program():
    nc = bacc.Bacc("TRN2", target_bir_lowering=False, debug=False, num_devices=8)
    dt_in = {}

    def din(name, shape, dt=F32):
        dt_in[name] = nc.dram_tensor(name, list(shape), dt, kind="ExternalInput")
        return dt_in[name]

    x1a_d = din("x1a", [128, T * 18 * 18], BF16)
    x1b_d = din("x1b", [128, T * 18 * 18], BF16)
    wx1a_d = din("wx1a", [128, 9 * 512], BF16)
    wx1bp_d = din("wx1bp", [128, 3 * 512], BF16)
    wx1bs_d = din("wx1bs", [128, 3 * 512], BF16)
    wh1_d = din("wh1", [128, 9 * 512], BF16)
    wx2_d = din("wx2", [128, 9 * 256], BF16)
    wh2p_d = din("wh2p", [128, 3 * 256], BF16)
    wh2s_d = din("wh2s", [128, 3 * 256], BF16)
    wx3p_d = din("wx3p", [128, 3 * 128], BF16)
    wx3s_d = din("wx3s", [128, 3 * 128], BF16)
    wh3p_d = din("wh3p", [128, 3 * 128], BF16)
    bnA1_d = din("bnA1", [128, 1])
    bnB1_d = din("bnB1", [128, 1])
    bnA2_d = din("bnA2", [64, 1])
    bnB2_d = din("bnB2", [64, 1])
    off_d = din("shard_off", [1, 1], I32)
    msk_d = din("rowmask", [96, 34])

    o3_d = nc.dram_tensor("o3", [T, 32, 16 * 64], BF16, kind="ExternalOutput")

    with tile.TileContext(nc) as tc:
        with tc.tile_pool(name="glob", bufs=1) as gp, \
             tc.tile_pool(name="dram", bufs=1, space="DRAM") as dp:
            stage2 = dp.tile([2 * 64, 82 * 68], BF16)   # 2-slot y2 ring
            # ---- persistent state ----
            h3A = gp.tile([128, 34 * 68], BF16)     # h | rep+1 | rep+2 | zeros
            h3B = gp.tile([128, 34 * 68], BF16)     # ping-pong partner of h3A
            S3 = gp.tile([64, 34 * 64], F32)        # c | tg (abs rows)
            wx3p = gp.tile([128, 3 * 128], BF16)
            wx3s = gp.tile([128, 3 * 128], BF16)
            wh3p = gp.tile([128, 3 * 128], BF16)
            offt = gp.tile([1, 1], I32)
            rowmask = gp.tile([96, 34], F32)
            half = gp.tile([128, 1], F32)
            x3a = gp.tile([128, 34 * 68], BF16)
            x3b = gp.tile([128, 34 * 68], BF16)
            y2sb = gp.tile([64, 64 * 68], BF16)     # one timestep of padded y2

            # spread initial loads across DMA queues (sync gets L1-critical)
            nc.gpsimd.dma_start(wx3p[:], wx3p_d[:])
            nc.gpsimd.dma_start(wx3s[:], wx3s_d[:])
            nc.gpsimd.dma_start(wh3p[:], wh3p_d[:])
            nc.gpsimd.dma_start(offt[:], off_d[:])
            nc.gpsimd.dma_start(rowmask[:], msk_d[:])
            nc.vector.memset(half[:], 0.5)
            nc.vector.memset(h3A[:], 0.0)
            nc.vector.memset(h3B[:], 0.0)
            nc.gpsimd.memset(S3[:], 0.0)
            nc.gpsimd.memset(x3a[:], 0.0)
            nc.gpsimd.memset(x3b[:], 0.0)
            nc.vector.memset(y2sb[:], 0.0)

            rv3 = nc.gpsimd.alloc_register("shardoff3")
            nc.gpsimd.reg_load(rv3, offt[0:1, 0:1])
            sv3 = nc.gpsimd.snap(rv3, donate=True, min_val=0, max_val=48)

            # zero the guard rows of both y2 stage slots (rows 0:9, 73:82)
            st2v = stage2[:].rearrange("(s c) (r w) -> s c r w", s=2, r=82)
            with tc.tile_pool(name="zp", bufs=1) as zp:
                zsrc = zp.tile([64, 9 * 68], BF16)
                nc.vector.memset(zsrc[:], 0.0)
                zv = zsrc[:].rearrange("c (r w) -> c r w", r=9)
                for s in range(2):
                    nc.scalar.dma_start(st2v[s, :, 0:9, :], zv[:])
                    nc.scalar.dma_start(st2v[s, :, 73:82, :], zv[:])

            with tc.tile_pool(name="p12", bufs=1) as p12:
                x2 = p12.tile([128, T * 34 * 34], BF16)
                bnA2 = p12.tile([64, 1], F32)
                bnB2 = p12.tile([64, 1], F32)
                # phase-2 persistent state, allocated before L1's tiles so the
                # loads overlap L1 compute instead of waiting on SBUF reuse
                wx2 = p12.tile([128, 9 * 256], BF16)
                wh2p = p12.tile([128, 3 * 256], BF16)
                wh2s = p12.tile([128, 3 * 256], BF16)
                h2A = p12.tile([128, 34 * 36], BF16)
                h2B = p12.tile([128, 34 * 36], BF16)
                S2 = p12.tile([128, 1024], F32)     # c | tg
                nc.gpsimd.dma_start(bnA2[:], bnA2_d[:])
                nc.gpsimd.dma_start(bnB2[:], bnB2_d[:])
                nc.gpsimd.dma_start(wx2[:], wx2_d[:])
                nc.gpsimd.dma_start(wh2p[:], wh2p_d[:])
                nc.gpsimd.dma_start(wh2s[:], wh2s_d[:])
                nc.gpsimd.memset(x2[:], 0.0)
                nc.gpsimd.memset(h2A[:], 0.0)
                nc.gpsimd.memset(h2B[:], 0.0)
                nc.gpsimd.memset(S2[:], 0.0)
                x2v = x2[:].rearrange("c (t r w) -> c t r w", t=T, r=34)

                # ================= Layer 1 =================
                with tc.tile_pool(name="l1w", bufs=1) as wp, \
                     tc.tile_pool(name="l1t", bufs=2) as tp, \
                     tc.tile_pool(name="ps1", bufs=2, space="PSUM") as pp:
                    x1a = wp.tile([128, T * 18 * 18], BF16)
                    x1b = wp.tile([128, T * 18 * 18], BF16)
                    wx1a = wp.tile([128, 9 * 512], BF16)
                    wx1bp = wp.tile([128, 3 * 512], BF16)
                    wx1bs = wp.tile([128, 3 * 512], BF16)
                    wh1 = wp.tile([128, 9 * 512], BF16)
                    bnA1 = wp.tile([128, 1], F32)
                    bnB1 = wp.tile([128, 1], F32)
                    h1 = wp.tile([128, 18 * 18], BF16)
                    c1 = wp.tile([128, 256], F32)
                    # L1-critical tensors on the sync queue, rest on scalar
                    nc.sync.dma_start(x1a[:], x1a_d[:])
                    nc.sync.dma_start(wx1a[:], wx1a_d[:])
                    nc.scalar.dma_start(x1b[:], x1b_d[:])
                    nc.scalar.dma_start(wx1bp[:], wx1bp_d[:])
                    nc.scalar.dma_start(wx1bs[:], wx1bs_d[:])
                    nc.scalar.dma_start(wh1[:], wh1_d[:])
                    nc.scalar.dma_start(bnA1[:], bnA1_d[:])
                    nc.scalar.dma_start(bnB1[:], bnB1_d[:])
                    nc.vector.memset(h1[:], 0.0)
                    nc.vector.memset(c1[:], 0.0)

                    x1av = x1a[:].rearrange("c (t r w) -> c t r w", t=T, r=18)
                    x1bv = x1b[:].rearrange("c (t r w) -> c t r w", t=T, r=18)
                    wx1av = wx1a[:].rearrange("c (k m) -> c k m", k=9)
                    wx1bpv = wx1bp[:].rearrange("c (k m) -> c k m", k=3)
                    wx1bsv = wx1bs[:].rearrange("c (k m) -> c k m", k=3)
                    wh1v = wh1[:].rearrange("c (k m) -> c k m", k=9)
                    h1v = h1[:].rearrange("c (r w) -> c r w", r=18)

                    for tpair in range(T // 2):
                        t0 = 2 * tpair
                        Gg = [pp.tile([128, 512], F32, tag=f"G{g}", name=f"G{g}_{tpair}")
                              for g in range(4)]
                        for g in range(4):
                            ms = slice(g * 128, g * 128 + 128)
                            first = True
                            for tau in range(9):
                                dy, dx = tau // 3 - 1, tau % 3 - 1
                                win = x1av[:, t0:t0 + 2, 1 + dy:17 + dy, 1 + dx:17 + dx]
                                nc.tensor.matmul(Gg[g][:], wx1av[:, tau, ms], win,
                                                 start=first, stop=False)
                                first = False
                            for dyi in range(3):
                                dy = dyi - 1
                                winp = x1bv[:, t0:t0 + 2, 1 + dy:17 + dy, 0:16]
                                nc.tensor.matmul(Gg[g][:], wx1bpv[:, dyi, ms], winp,
                                                 start=False, stop=False)
                            for dyi in range(3):
                                dy = dyi - 1
                                wins = x1bv[:, t0:t0 + 2, 1 + dy:17 + dy, 2:18]
                                nc.tensor.matmul(Gg[g][:], wx1bsv[:, dyi, ms], wins,
                                                 start=False, stop=False)
                        for p in range(2):
                            t = t0 + p
                            col = slice(p * 256, p * 256 + 256)
                            if not (tpair == 0 and p == 0):   # h==0 at t=0
                                for g in (2, 0, 1, 3):        # g-gate first: tanh heads the chain
                                    ms = slice(g * 128, g * 128 + 128)
                                    for tau in range(9):
                                        dy, dx = tau // 3 - 1, tau % 3 - 1
                                        win = h1v[:, 1 + dy:17 + dy, 1 + dx:17 + dx]
                                        nc.tensor.matmul(Gg[g][:, col], wh1v[:, tau, ms], win,
                                                         start=False, stop=(p == 1 and tau == 8))
                            # elementwise: gate order i,f,g,o in Gg[0..3]
                            tg1 = tp.tile([128, 256], F32, tag="tg1")
                            Uf = tp.tile([128, 256], F32, tag="Uf")
                            Ui = tp.tile([128, 256], F32, tag="Ui")
                            Uo = tp.tile([128, 256], F32, tag="Uo")
                            Pf = tp.tile([128, 256], F32, tag="Pf")
                            Pi = tp.tile([128, 256], F32, tag="Pi")
                            tc1 = tp.tile([128, 256], F32, tag="tc1")
                            y1 = tp.tile([128, 256], BF16, tag="y1")
                            nc.scalar.activation(tg1[:], Gg[2][:, col], AF.Tanh)
                            nc.scalar.activation(Ui[:], Gg[0][:, col], AF.Relu, bias=half[:, 0:1], scale=0.2)
                            nc.scalar.activation(Uo[:], Gg[3][:, col], AF.Relu, bias=half[:, 0:1], scale=0.2)
                            if tpair == 0 and p == 0:         # c==0: c1 = i_hat*tg
                                nc.vector.scalar_tensor_tensor(
                                    out=c1[:], in0=Ui[:], scalar=1.0, in1=tg1[:],
                                    op0=OP.min, op1=OP.mult)
                            else:
                                nc.scalar.activation(Uf[:], Gg[1][:, col], AF.Relu, bias=half[:, 0:1], scale=0.2)
                                nc.vector.scalar_tensor_tensor(
                                    out=Pf[:], in0=Uf[:], scalar=1.0, in1=c1[:],
                                    op0=OP.min, op1=OP.mult)
                                nc.vector.scalar_tensor_tensor(
                                    out=Pi[:], in0=Ui[:], scalar=1.0, in1=tg1[:],
                                    op0=OP.min, op1=OP.mult)
                                nc.vector.tensor_tensor(out=c1[:], in0=Pf[:], in1=Pi[:], op=OP.add)
                            nc.scalar.activation(tc1[:], c1[:], AF.Tanh)
                            nc.vector.scalar_tensor_tensor(
                                out=h1v[:, 1:17, 1:17], in0=Uo[:], scalar=1.0, in1=tc1[:],
                                op0=OP.min, op1=OP.mult)
                            # BN + 2x upsample -> x2[t] interior
                            nc.scalar.activation(y1[:], h1v[:, 1:17, 1:17],
                                                 AF.Identity, bias=bnB1[:, 0:1], scale=bnA1[:, 0:1])
                            ups = y1[:].rearrange("c (r w) -> c r w", r=16).unsqueeze(2).unsqueeze(4) \
                                .broadcast_to([128, 16, 2, 16, 2])
                            nc.vector.tensor_copy(x2v[:, t, 1:33, 1:33], ups)

                # ======== Phase 2: interleaved Layer 2 + Layer 3 ========
                with tc.tile_pool(name="p2t", bufs=2) as tp2, \
                     tc.tile_pool(name="ps2", bufs=1, space="PSUM") as pp2, \
                     tc.tile_pool(name="ps3", bufs=2, space="PSUM") as pp3:
                    wx2v = wx2[:].rearrange("c (k m) -> c k m", k=9)
                    wh2pv = wh2p[:].rearrange("c (k m) -> c k m", k=3)
                    wh2sv = wh2s[:].rearrange("c (k m) -> c k m", k=3)
                    h2Av = h2A[:].rearrange("c (r w) -> c r w", r=34)
                    h2Bv = h2B[:].rearrange("c (r w) -> c r w", r=34)
                    y2v = y2sb[:].rearrange("c (r w) -> c r w", r=64)
                    wx3pv = wx3p[:].rearrange("c (k m) -> c k m", k=3)
                    wx3sv = wx3s[:].rearrange("c (k m) -> c k m", k=3)
                    wh3pv = wh3p[:].rearrange("c (k m) -> c k m", k=3)
                    h3Av = h3A[:].rearrange("c (r w) -> c r w", r=34)
                    h3Bv = h3B[:].rearrange("c (r w) -> c r w", r=34)
                    o3v = o3_d[:].rearrange("t c (r w) -> t c r w", r=16)

                    for t in range(T):
                        # ---------- L2 timestep t ----------
                        h2pv = h2Av if t % 2 == 0 else h2Bv   # h(t-1)
                        h2cv = h2Bv if t % 2 == 0 else h2Av   # h(t)
                        GA = pp2.tile([128, 1024], F32, tag="GA")  # f | i
                        GB = pp2.tile([128, 1024], F32, tag="GB")  # o | g
                        for blk, gout in ((0, GA), (1, GB)):
                            ms = slice(blk * 128, blk * 128 + 128)
                            for pc in range(2):
                                out = gout[:, pc * 512:(pc + 1) * 512]
                                r0 = 1 + 16 * pc
                                first = True
                                for tau in range(9):
                                    dy, dx = tau // 3 - 1, tau % 3 - 1
                                    win = x2v[:, t, r0 + dy:r0 + 16 + dy, 1 + dx:33 + dx]
                                    nc.tensor.matmul(out, wx2v[:, tau, ms], win,
                                                     start=first, stop=(t == 0 and tau == 8))
                                    first = False
                                if t == 0:
                                    continue              # h==0 at t=0
                                for dyi in range(3):
                                    dy = dyi - 1
                                    winp = h2pv[:, r0 + dy:r0 + 16 + dy, 1:33]
                                    nc.tensor.matmul(out, wh2pv[:, dyi, ms], winp,
                                                     start=False, stop=False)
                                for dyi in range(3):
                                    dy = dyi - 1
                                    wins = h2pv[:, r0 + dy:r0 + 16 + dy, 3:35]
                                    nc.tensor.matmul(out, wh2sv[:, dyi, ms], wins,
                                                     start=False, stop=(dyi == 2))
                        UA = tp2.tile([128, 1024], F32, tag="UA")
                        Uo2 = tp2.tile([64, 1024], F32, tag="Uo2")
                        P2f = tp2.tile([64, 1024], F32, tag="P2f")
                        P2i = tp2.tile([64, 1024], F32, tag="P2i")
                        tc2 = tp2.tile([64, 1024], F32, tag="tc2")
                        y2f = tp2.tile([64, 1024], BF16, tag="y2f")
                        nc.scalar.activation(S2[64:128, :], GB[64:128, :], AF.Tanh)
                        nc.scalar.activation(UA[:], GA[:], AF.Relu, bias=half[:, 0:1], scale=0.2)
                        nc.scalar.activation(Uo2[:], GB[0:64, :], AF.Relu, bias=half[0:64, 0:1], scale=0.2)
                        if t == 0:                        # c==0: c = i_hat*tg
                            nc.vector.scalar_tensor_tensor(
                                out=S2[0:64, :], in0=UA[64:128, :], scalar=1.0, in1=S2[64:128, :],
                                op0=OP.min, op1=OP.mult)
                        else:
                            nc.vector.scalar_tensor_tensor(
                                out=P2f[:], in0=UA[0:64, :], scalar=1.0, in1=S2[0:64, :],
                                op0=OP.min, op1=OP.mult)
                            nc.vector.scalar_tensor_tensor(
                                out=P2i[:], in0=UA[64:128, :], scalar=1.0, in1=S2[64:128, :],
                                op0=OP.min, op1=OP.mult)
                            nc.vector.tensor_tensor(out=S2[0:64, :], in0=P2f[:], in1=P2i[:], op=OP.add)
                        if t < T - 1:
                            nc.scalar.activation(tc2[:], S2[0:64, :], AF.Tanh)
                            nc.vector.scalar_tensor_tensor(
                                out=h2cv[0:64, 1:33, 2:34], in0=Uo2[:], scalar=1.0, in1=tc2[:],
                                op0=OP.min, op1=OP.mult)
                            nc.vector.tensor_copy(h2cv[64:128, 1:33, 1:34], h2cv[0:64, 1:33, 2:35])
                            # BN + upsample -> y2sb
                            nc.scalar.activation(y2f[:], h2cv[0:64, 1:33, 2:34],
                                                 AF.Identity, bias=bnB2[:, 0:1], scale=bnA2[:, 0:1])
                            ups2 = y2f[:].rearrange("c (r w) -> c r w", r=32).unsqueeze(2).unsqueeze(4) \
                                .broadcast_to([64, 32, 2, 32, 2])
                            nc.vector.tensor_copy(y2v[:, :, 2:66], ups2)
                            nc.scalar.dma_start(st2v[t % 2, :, 9:73, :], y2v[:])
                        else:
                            # last step: chain fully exposed -> pipeline in halves,
                            # skip the h2 replica (no t+1 consumes it)
                            for pc in range(2):
                                cs = slice(pc * 512, (pc + 1) * 512)
                                rr = slice(1 + 16 * pc, 17 + 16 * pc)
                                tc2c = tp2.tile([64, 512], F32, tag=f"tc2c{pc}")
                                y2fc = tp2.tile([64, 512], BF16, tag=f"y2fc{pc}")
                                nc.scalar.activation(tc2c[:], S2[0:64, cs], AF.Tanh)
                                nc.vector.scalar_tensor_tensor(
                                    out=h2cv[0:64, rr, 2:34], in0=Uo2[:, cs], scalar=1.0,
                                    in1=tc2c[:], op0=OP.min, op1=OP.mult)
                                nc.scalar.activation(y2fc[:], h2cv[0:64, rr, 2:34],
                                                     AF.Identity, bias=bnB2[:, 0:1], scale=bnA2[:, 0:1])
                                upsc = y2fc[:].rearrange("c (r w) -> c r w", r=16).unsqueeze(2).unsqueeze(4) \
                                    .broadcast_to([64, 16, 2, 32, 2])
                                nc.vector.tensor_copy(y2v[:, 32 * pc:32 * pc + 32, 2:66], upsc)
                                nc.scalar.dma_start(st2v[t % 2, :, 9 + 32 * pc:41 + 32 * pc, :],
                                                    y2v[:, 32 * pc:32 * pc + 32, :])

                        # ---------- L3 timestep t ----------
                        a, b_ = 2 + t, 32 - t   # backward-induced need: out rows 9:25 at t=7
                        rows = b_ - a
                        hpv = h3Av if t % 2 == 0 else h3Bv   # h(t-1), read by h-convs
                        hcv = h3Bv if t % 2 == 0 else h3Av   # h(t), written by elementwise
                        x3 = (x3a if t % 2 == 0 else x3b)
                        x3v = x3[:].rearrange("c (r w) -> c r w", r=34)
                        nc.gpsimd.dma_start(x3v[0:64, :, :],
                                            st2v[t % 2, :, bass.ds(sv3, 34), :])
                        nc.gpsimd.dma_start(x3v[64:128, :, 0:67],
                                            st2v[t % 2, :, bass.ds(sv3, 34), 1:68])
                        nblk = (rows + 7) // 8
                        nb0 = (nblk + 1) // 2
                        mid = min(b_, a + 8 * nb0)
                        for (ca, cb) in ((a, mid), (mid, b_)):
                            crows = cb - ca
                            G3c = pp3.tile([128, 1024], F32, tag="G3", name=f"G3_{t}_{ca}")
                            npc = (crows + 7) // 8
                            for pc in range(npc):
                                r0 = ca + 8 * pc
                                r1 = min(cb, r0 + 8)
                                out = G3c[:, (r0 - ca) * 64:(r1 - ca) * 64]
                                first = True
                                for dyi in range(3):
                                    dy = dyi - 1
                                    winp = x3v[:, r0 + dy:r1 + dy, 1:65]
                                    nc.tensor.matmul(out, wx3pv[:, dyi, :], winp,
                                                     start=first, stop=False)
                                    first = False
                                    wins = x3v[:, r0 + dy:r1 + dy, 3:67]
                                    nc.tensor.matmul(out, wx3sv[:, dyi, :], wins,
                                                     start=False, stop=(t == 0 and dyi == 2))
                                if t == 0:
                                    continue              # h==0 at t=0
                                for dyi in range(3):
                                    dy = dyi - 1
                                    winh = hpv[:, r0 + dy:r1 + dy, 1:65]
                                    nc.tensor.matmul(out, wh3pv[:, dyi, :], winh,
                                                     start=False, stop=(dyi == 2))
                            Fc = crows * 64
                            sroi = slice(ca * 64, cb * 64)
                            U3 = tp2.tile([96, 1024], F32, tag="U3")
                            Um = tp2.tile([32, 1024], F32, tag="Um")
                            P3f = tp2.tile([32, 1024], F32, tag="P3f")
                            P3i = tp2.tile([32, 1024], F32, tag="P3i")
                            tc3 = tp2.tile([32, 1024], F32, tag="tc3")
                            nc.scalar.activation(S3[32:64, sroi], G3c[96:128, 0:Fc], AF.Tanh)
                            nc.scalar.activation(U3[:, 0:Fc], G3c[0:96, 0:Fc], AF.Relu,
                                                 bias=half[0:96, 0:1], scale=0.2)
                            mview = rowmask[64:96, ca:cb].unsqueeze(2).broadcast_to([32, crows, 64])
                            nc.vector.tensor_tensor(
                                out=Um[:, 0:Fc].rearrange("c (r w) -> c r w", w=64),
                                in0=U3[64:96, 0:Fc].rearrange("c (r w) -> c r w", w=64),
                                in1=mview, op=OP.mult)
                            if t == 0:                    # c==0: c = i_hat*tg
                                nc.vector.scalar_tensor_tensor(
                                    out=S3[0:32, sroi], in0=U3[32:64, 0:Fc], scalar=1.0,
                                    in1=S3[32:64, sroi], op0=OP.min, op1=OP.mult)
                            else:
                                nc.vector.scalar_tensor_tensor(
                                    out=P3f[:, 0:Fc], in0=U3[0:32, 0:Fc], scalar=1.0,
                                    in1=S3[0:32, sroi], op0=OP.min, op1=OP.mult)
                                nc.vector.scalar_tensor_tensor(
                                    out=P3i[:, 0:Fc], in0=U3[32:64, 0:Fc], scalar=1.0,
                                    in1=S3[32:64, sroi], op0=OP.min, op1=OP.mult)
                                nc.vector.tensor_tensor(out=S3[0:32, sroi], in0=P3f[:, 0:Fc],
                                                        in1=P3i[:, 0:Fc], op=OP.add)
                            nc.scalar.activation(tc3[:, 0:Fc], S3[0:32, sroi], AF.Tanh)
                            nc.vector.tensor_tensor(
                                out=hcv[0:32, ca:cb, 2:66],
                                in0=Um[:, 0:Fc].rearrange("c (r w) -> c r w", w=64),
                                in1=tc3[:, 0:Fc].rearrange("c (r w) -> c r w", w=64), op=OP.mult)
                            if t < T - 1:   # replicas only feed the next step's h-convs
                                nc.vector.tensor_copy(hcv[32:64, ca:cb, 1:65], hcv[0:32, ca:cb, 2:66])
                                nc.vector.tensor_copy(hcv[64:96, ca:cb, 1:65], hcv[0:32, ca:cb, 3:67])
                        nc.sync.dma_start(o3v[t], hcv[0:32, 9:25, 2:66])

    nc.compile()
    return nc


def _prep_inputs(inputs):
    """Build the 8 per-core input maps from the full problem inputs."""
    x = np.asarray(inputs["x"], np.float32)
    W9 = {}
    for l in (1, 2, 3):
        W9[f"x{l}"] = np.asarray(inputs[f"Wx{l}"], np.float32).reshape(9, *inputs[f"Wx{l}"].shape[2:])
        W9[f"h{l}"] = np.asarray(inputs[f"Wh{l}"], np.float32).reshape(9, *inputs[f"Wh{l}"].shape[2:])

    def bn(l, C):
        g = np.asarray(inputs[f"g{l}"], np.float32)
        be = np.asarray(inputs[f"be{l}"], np.float32)
        mm = np.asarray(inputs[f"mm{l}"], np.float32)
        mv = np.asarray(inputs[f"mv{l}"], np.float32)
        A = g / np.sqrt(mv + EPS)
        Bc = be - mm * A
        return A.reshape(C, 1), Bc.reshape(C, 1)

    A1, B1 = bn(1, 128)
    A2, B2 = bn(2, 64)
    A3, B3 = bn(3, 32)

    def padk(a, k=128):
        return np.concatenate([a, np.zeros((k - a.shape[0],) + a.shape[1:], a.dtype)], axis=0)

    wx1 = W9["x1"]  # [9, 192, 512]
    wx1a = wx1[:, 0:128].transpose(1, 0, 2).reshape(128, -1)
    wx1bp = np.concatenate(
        [np.concatenate([wx1[3 * dy + 0, 128:192], wx1[3 * dy + 1, 128:192]], axis=0)[None]
         for dy in range(3)], axis=0).transpose(1, 0, 2).reshape(128, -1)
    wx1bs = padk(wx1[[2, 5, 8], 128:192].transpose(1, 0, 2).reshape(64, -1))
    wh1 = W9["h1"].transpose(1, 0, 2).reshape(128, -1)

    p2 = np.r_[64:128, 0:64, 192:256, 128:192]  # [f,i,o,g]
    wx2 = W9["x2"][:, :, p2].transpose(1, 0, 2).reshape(128, -1)
    wh2 = W9["h2"][:, :, p2]  # [9, 64, 256]
    wh2p = np.concatenate(
        [np.concatenate([wh2[3 * dy + 0], wh2[3 * dy + 1]], axis=0)[None]
         for dy in range(3)], axis=0).transpose(1, 0, 2).reshape(128, -1)
    wh2s = padk(wh2[[2, 5, 8]].transpose(1, 0, 2).reshape(64, -1))

    p3 = np.r_[32:64, 0:32, 96:128, 64:96]  # [f,i,o,g]
    wx3 = W9["x3"][:, :, p3]  # [9, 64, 128]
    wx3p = np.concatenate(
        [np.concatenate([wx3[3 * dy + 0], wx3[3 * dy + 1]], axis=0)[None]
         for dy in range(3)], axis=0).transpose(1, 0, 2).reshape(128, -1)
    wx3s = padk(wx3[[2, 5, 8]].transpose(1, 0, 2).reshape(64, -1))
    wh3 = W9["h3"][:, :, p3]  # [9, 32, 128]
    wh3p = padk(np.concatenate(
        [np.concatenate([wh3[3 * dy + 0], wh3[3 * dy + 1], wh3[3 * dy + 2]], axis=0)[None]
         for dy in range(3)], axis=0).transpose(1, 0, 2).reshape(96, -1))

    shared = dict(wx1a=wx1a, wx1bp=wx1bp, wx1bs=wx1bs, wh1=wh1,
                  wx2=wx2, wh2p=wh2p, wh2s=wh2s,
                  wx3p=wx3p, wx3s=wx3s, wh3p=wh3p,
                  bnA1=A1, bnB1=B1, bnA2=A2, bnB2=B2)
    import ml_dtypes
    bf = ml_dtypes.bfloat16
    bf_keys = {"wx1a", "wx1bp", "wx1bs", "wh1", "wx2", "wh2p", "wh2s",
               "wx3p", "wx3s", "wh3p"}
    shared = {k: (np.ascontiguousarray(v).astype(bf) if k in bf_keys
                  else np.ascontiguousarray(v, np.float32))
              for k, v in shared.items()}

    maps = []
    for core in range(8):
        b = core // 4
        q = core % 4
        xp = np.zeros((T, 192, 18, 18), np.float32)
        xp[:, :, 1:17, 1:17] = x[b].transpose(0, 3, 1, 2)
        x1a = np.ascontiguousarray(xp[:, 0:128].transpose(1, 0, 2, 3).reshape(128, -1))
        xb = xp[:, 128:192]
        xbs = np.zeros_like(xb)
        xbs[..., :-1] = xb[..., 1:]
        x1b = np.ascontiguousarray(
            np.concatenate([xb, xbs], axis=1).transpose(1, 0, 2, 3).reshape(128, -1))
        m = dict(shared)
        m["x1a"] = x1a.astype(bf)
        m["x1b"] = x1b.astype(bf)
        m["shard_off"] = np.array([[16 * q]], np.int32)
        gl = 16 * q - 9 + np.arange(34)
        mask = ((gl >= 0) & (gl < 64)).astype(np.float32)
        m["rowmask"] = np.ascontiguousarray(np.broadcast_to(mask, (96, 34)))
        maps.append(m)
    return maps, (A3.reshape(32), B3.reshape(32))


def kernel(**inputs):
    global _PROG
    if _PROG is None:
        _PROG = build_program()
    nc = _PROG
    maps, (A3, B3) = _prep_inputs(inputs)

    trace = bool(os.environ.get("KERNEL_TRACE"))
    kw = {}
    if trace and _install_trace_hook():
        kw = dict(trace=True, trace_cores=[0])
    res = run_bass_kernel_spmd(nc, maps, core_ids=list(range(8)), **kw)
    if trace:
        kernel.last_exec_ns = res.exec_time_ns

    # assemble: core b*4+q provides H3 rows [16q, 16q+16)
    h3 = np.zeros((B, T, 32, 64, 64), np.float32)
    for core in range(8):
        b, q = core // 4, core % 4
        h3[b, :, :, 16 * q:16 * q + 16, :] = \
            res.results[core]["o3"].astype(np.float32).reshape(T, 32, 16, 64)
    y = h3 * A3[None, None, :, None, None] + B3[None, None, :, None, None]
    y = np.repeat(np.repeat(y, 2, axis=3), 2, axis=4)  # [B,T,32,128,128]
    return np.ascontiguousarray(y.transpose(0, 1, 3, 4, 2))


kernel.last_exec_ns = None


# revision 57
# speedup vs baseline: 1.0016x; 1.0016x over previous
"""Trainium2 Bass kernel for nn_DecCLSTMBlock (3x stacked ConvLSTM2D + BN + 2x2 upsample).

Sharding: 8 cores = 2 batch groups x 4 H-shards.
  - L1 (16x16, Cout=128) and L2 (32x32, Cout=64) computed fully per core
    (replicated within the batch group).
  - L3 (64x64, Cout=32) sharded 4 ways over H with a shrinking-halo
    overcompute domain of 34 rows -> no inter-core communication.
  - L2 and L3 are interleaved per timestep: L3's elementwise-heavy chain
    overlaps L2's matmul-heavy stream. y2 stays in SBUF (no DRAM staging).
  - Final BN + upsample of layer 3 done host-side in numpy.
All conv weights are zero-padded to K=128 partitions so every LDWEIGHTS has
identical full-array geometry (lets the PE pull weight loads ahead of
in-flight matmuls). LSTM cell elementwise uses scalar_tensor_tensor fusion:
(U min 1) * v does hard_sigmoid clamp + gate multiply in one op.
Matmuls run in bf16 with fp32 PSUM accumulation (cell state c stays fp32).
Assumes bias vectors b1..b3 are zero (they are, per the problem spec).
"""
import os
import sys
import types

sys.path.insert(0, "/opt/trn_rl_repo")

import numpy as np

import concourse.bass as bass
import concourse.tile as tile
from concourse import bacc, mybir
from concourse.bass_utils import run_bass_kernel_spmd

F32 = mybir.dt.float32
BF16 = mybir.dt.bfloat16
I32 = mybir.dt.int32
AF = mybir.ActivationFunctionType
OP = mybir.AluOpType

B, T = 2, 8
EPS = 1e-3

_PROG = None


def _install_trace_hook():
    try:
        import antenv
        if 'antenv.axon_hooks' not in sys.modules:
            mod = types.ModuleType('antenv.axon_hooks')
            mod._hook = None
            def _set(h):
                mod._hook = h
            def _get():
                return mod._hook
            mod.set_axon_ntff_profile_hook = _set
            mod.get_axon_ntff_profile_hook = _get
            sys.modules['antenv.axon_hooks'] = mod
            antenv.axon_hooks = mod
            from trn_agent_boot.trn_boot import _ntff_profile_via_ctypes
            mod.set_axon_ntff_profile_hook(
                _ntff_profile_via_ctypes('/opt/axon/libaxon_pjrt.so'))
        import concourse.bass_utils as bu
        bu.upload_artifacts = lambda tmpdir: "local://" + tmpdir
        return True
    except Exception:
        return False


def build_program():
    nc = bacc.Bacc("TRN2", target_bir_lowering=False, debug=False, num_devices=8)
    dt_in = {}

    def din(name, shape, dt=F32):
        dt_in[name] = nc.dram_tensor(name, list(shape), dt, kind="ExternalInput")
        return dt_in[name]

    x1a_d = din("x1a", [128, T * 18 * 18], BF16)
    x1b_d = din("x1b", [128, T * 18 * 18], BF16)
    wx1a_d = din("wx1a", [128, 9 * 512], BF16)
    wx1bp_d = din("wx1bp", [128, 3 * 512], BF16)
    wx1bs_d = din("wx1bs", [128, 3 * 512], BF16)
    wh1_d = din("wh1", [128, 9 * 512], BF16)
    wx2_d = din("wx2", [128, 9 * 256], BF16)
    wh2p_d = din("wh2p", [128, 3 * 256], BF16)
    wh2s_d = din("wh2s", [128, 3 * 256], BF16)
    wx3p_d = din("wx3p", [128, 3 * 128], BF16)
    wx3s_d = din("wx3s", [128, 3 * 128], BF16)
    wh3p_d = din("wh3p", [128, 3 * 128], BF16)
    bnA1_d = din("bnA1", [128, 1])
    bnB1_d = din("bnB1", [128, 1])
    bnA2_d = din("bnA2", [64, 1])
    bnB2_d = din("bnB2", [64, 1])
    off_d = din("shard_off", [1, 1], I32)
    msk_d = din("rowmask", [96, 34])

    o3_d = nc.dram_tensor("o3", [T, 32, 16 * 64], BF16, kind="ExternalOutput")

    with tile.TileContext(nc) as tc:
        with tc.tile_pool(name="glob", bufs=1) as gp, \
             tc.tile_pool(name="dram", bufs=1, space="DRAM") as dp:
            stage2 = dp.tile([2 * 64, 82 * 68], BF16)   # 2-slot y2 ring
            # ---- persistent state ----
            h3A = gp.tile([128, 34 * 68], BF16)     # h | rep+1 | rep+2 | zeros
            h3B = gp.tile([128, 34 * 68], BF16)     # ping-pong partner of h3A
            S3 = gp.tile([64, 34 * 64], F32)        # c | tg (abs rows)
            wx3p = gp.tile([128, 3 * 128], BF16)
            wx3s = gp.tile([128, 3 * 128], BF16)
            wh3p = gp.tile([128, 3 * 128], BF16)
            offt = gp.tile([1, 1], I32)
            rowmask = gp.tile([96, 34], F32)
            half = gp.tile([128, 1], F32)
            x3a = gp.tile([128, 34 * 68], BF16)
            x3b = gp.tile([128, 34 * 68], BF16)
            y2sb = gp.tile([64, 64 * 68], BF16)     # one timestep of padded y2

            # spread initial loads across DMA queues (sync gets L1-critical)
            nc.gpsimd.dma_start(wx3p[:], wx3p_d[:])
            nc.gpsimd.dma_start(wx3s[:], wx3s_d[:])
            nc.gpsimd.dma_start(wh3p[:], wh3p_d[:])
            nc.gpsimd.dma_start(offt[:], off_d[:])
            nc.gpsimd.dma_start(rowmask[:], msk_d[:])
            nc.vector.memset(half[:], 0.5)
            nc.vector.memset(h3A[:], 0.0)
            nc.vector.memset(h3B[:], 0.0)
            nc.gpsimd.memset(S3[:], 0.0)
            nc.gpsimd.memset(x3a[:], 0.0)
            nc.gpsimd.memset(x3b[:], 0.0)
            nc.vector.memset(y2sb[:], 0.0)

            rv3 = nc.gpsimd.alloc_register("shardoff3")
            nc.gpsimd.reg_load(rv3, offt[0:1, 0:1])
            sv3 = nc.gpsimd.snap(rv3, donate=True, min_val=0, max_val=48)

            # zero the guard rows of both y2 stage slots (rows 0:9, 73:82)
            st2v = stage2[:].rearrange("(s c) (r w) -> s c r w", s=2, r=82)
            with tc.tile_pool(name="zp", bufs=1) as zp:
                zsrc = zp.tile([64, 9 * 68], BF16)
                nc.vector.memset(zsrc[:], 0.0)
                zv = zsrc[:].rearrange("c (r w) -> c r w", r=9)
                for s in range(2):
                    nc.scalar.dma_start(st2v[s, :, 0:9, :], zv[:])
                    nc.scalar.dma_start(st2v[s, :, 73:82, :], zv[:])

            with tc.tile_pool(name="p12", bufs=1) as p12:
                x2 = p12.tile([128, T * 34 * 34], BF16)
                bnA2 = p12.tile([64, 1], F32)
                bnB2 = p12.tile([64, 1], F32)
                # phase-2 persistent state, allocated before L1's tiles so the
                # loads overlap L1 compute instead of waiting on SBUF reuse
                wx2 = p12.tile([128, 9 * 256], BF16)
                wh2p = p12.tile([128, 3 * 256], BF16)
                wh2s = p12.tile([128, 3 * 256], BF16)
                h2A = p12.tile([128, 34 * 36], BF16)
                h2B = p12.tile([128, 34 * 36], BF16)
                S2 = p12.tile([128, 1024], F32)     # c | tg
                nc.gpsimd.dma_start(bnA2[:], bnA2_d[:])
                nc.gpsimd.dma_start(bnB2[:], bnB2_d[:])
                nc.gpsimd.dma_start(wx2[:], wx2_d[:])
                nc.gpsimd.dma_start(wh2p[:], wh2p_d[:])
                nc.gpsimd.dma_start(wh2s[:], wh2s_d[:])
                nc.gpsimd.memset(x2[:], 0.0)
                nc.gpsimd.memset(h2A[:], 0.0)
                nc.gpsimd.memset(h2B[:], 0.0)
                nc.gpsimd.memset(S2[:], 0.0)
                x2v = x2[:].rearrange("c (t r w) -> c t r w", t=T, r=34)

                # ================= Layer 1 =================
                with tc.tile_pool(name="l1w", bufs=1) as wp, \
                     tc.tile_pool(name="l1t", bufs=2) as tp, \
                     tc.tile_pool(name="ps1", bufs=2, space="PSUM") as pp:
                    x1a = wp.tile([128, T * 18 * 18], BF16)
                    x1b = wp.tile([128, T * 18 * 18], BF16)
                    wx1a = wp.tile([128, 9 * 512], BF16)
                    wx1bp = wp.tile([128, 3 * 512], BF16)
                    wx1bs = wp.tile([128, 3 * 512], BF16)
                    wh1 = wp.tile([128, 9 * 512], BF16)
                    bnA1 = wp.tile([128, 1], F32)
                    bnB1 = wp.tile([128, 1], F32)
                    h1 = wp.tile([128, 18 * 18], BF16)
                    c1 = wp.tile([128, 256], F32)
                    # L1-critical tensors on the sync queue, rest on scalar
                    nc.sync.dma_start(x1a[:], x1a_d[:])
                    nc.sync.dma_start(wx1a[:], wx1a_d[:])
                    nc.scalar.dma_start(x1b[:], x1b_d[:])
                    nc.scalar.dma_start(wx1bp[:], wx1bp_d[:])
                    nc.scalar.dma_start(wx1bs[:], wx1bs_d[:])
                    nc.scalar.dma_start(wh1[:], wh1_d[:])
                    nc.scalar.dma_start(bnA1[:], bnA1_d[:])
                    nc.scalar.dma_start(bnB1[:], bnB1_d[:])
                    nc.vector.memset(h1[:], 0.0)
                    nc.vector.memset(c1[:], 0.0)

                    x1av = x1a[:].rearrange("c (t r w) -> c t r w", t=T, r=18)
                    x1bv = x1b[:].rearrange("c (t r w) -> c t r w", t=T, r=18)
                    wx1av = wx1a[:].rearrange("c (k m) -> c k m", k=9)
                    wx1bpv = wx1bp[:].rearrange("c (k m) -> c k m", k=3)
                    wx1bsv = wx1bs[:].rearrange("c (k m) -> c k m", k=3)
                    wh1v = wh1[:].rearrange("c (k m) -> c k m", k=9)
                    h1v = h1[:].rearrange("c (r w) -> c r w", r=18)

                    for tpair in range(T // 2):
                        t0 = 2 * tpair
                        Gg = [pp.tile([128, 512], F32, tag=f"G{g}", name=f"G{g}_{tpair}")
                              for g in range(4)]
                        for g in range(4):
                            ms = slice(g * 128, g * 128 + 128)
                            first = True
                            for tau in range(9):
                                dy, dx = tau // 3 - 1, tau % 3 - 1
                                win = x1av[:, t0:t0 + 2, 1 + dy:17 + dy, 1 + dx:17 + dx]
                                nc.tensor.matmul(Gg[g][:], wx1av[:, tau, ms], win,
                                                 start=first, stop=False)
                                first = False
                            for dyi in range(3):
                                dy = dyi - 1
                                winp = x1bv[:, t0:t0 + 2, 1 + dy:17 + dy, 0:16]
                                nc.tensor.matmul(Gg[g][:], wx1bpv[:, dyi, ms], winp,
                                                 start=False, stop=False)
                            for dyi in range(3):
                                dy = dyi - 1
                                wins = x1bv[:, t0:t0 + 2, 1 + dy:17 + dy, 2:18]
                                nc.tensor.matmul(Gg[g][:], wx1bsv[:, dyi, ms], wins,
                                                 start=False, stop=False)
                        for p in range(2):
                            t = t0 + p
                            col = slice(p * 256, p * 256 + 256)
                            if not (tpair == 0 and p == 0):   # h==0 at t=0
                                for g in (2, 0, 1, 3):        # g-gate first: tanh heads the chain
                                    ms = slice(g * 128, g * 128 + 128)
                                    for tau in range(9):
                                        dy, dx = tau // 3 - 1, tau % 3 - 1
                                        win = h1v[:, 1 + dy:17 + dy, 1 + dx:17 + dx]
                                        nc.tensor.matmul(Gg[g][:, col], wh1v[:, tau, ms], win,
                                                         start=False, stop=(p == 1 and tau == 8))
                            # elementwise: gate order i,f,g,o in Gg[0..3]
                            tg1 = tp.tile([128, 256], F32, tag="tg1")
                            Uf = tp.tile([128, 256], F32, tag="Uf")
                            Ui = tp.tile([128, 256], F32, tag="Ui")
                            Uo = tp.tile([128, 256], F32, tag="Uo")
                            Pf = tp.tile([128, 256], F32, tag="Pf")
                            Pi = tp.tile([128, 256], F32, tag="Pi")
                            tc1 = tp.tile([128, 256], F32, tag="tc1")
                            y1 = tp.tile([128, 256], BF16, tag="y1")
                            nc.scalar.activation(tg1[:], Gg[2][:, col], AF.Tanh)
                            nc.scalar.activation(Ui[:], Gg[0][:, col], AF.Relu, bias=half[:, 0:1], scale=0.2)
                            nc.scalar.activation(Uo[:], Gg[3][:, col], AF.Relu, bias=half[:, 0:1], scale=0.2)
                            if tpair == 0 and p == 0:         # c==0: c1 = i_hat*tg
                                nc.vector.scalar_tensor_tensor(
                                    out=c1[:], in0=Ui[:], scalar=1.0, in1=tg1[:],
                                    op0=OP.min, op1=OP.mult)
                            else:
                                nc.scalar.activation(Uf[:], Gg[1][:, col], AF.Relu, bias=half[:, 0:1], scale=0.2)
                                nc.vector.scalar_tensor_tensor(
                                    out=Pf[:], in0=Uf[:], scalar=1.0, in1=c1[:],
                                    op0=OP.min, op1=OP.mult)
                                nc.vector.scalar_tensor_tensor(
                                    out=Pi[:], in0=Ui[:], scalar=1.0, in1=tg1[:],
                                    op0=OP.min, op1=OP.mult)
                                nc.vector.tensor_tensor(out=c1[:], in0=Pf[:], in1=Pi[:], op=OP.add)
                            nc.scalar.activation(tc1[:], c1[:], AF.Tanh)
                            nc.vector.scalar_tensor_tensor(
                                out=h1v[:, 1:17, 1:17], in0=Uo[:], scalar=1.0, in1=tc1[:],
                                op0=OP.min, op1=OP.mult)
                            # BN + 2x upsample -> x2[t] interior
                            nc.scalar.activation(y1[:], h1v[:, 1:17, 1:17],
                                                 AF.Identity, bias=bnB1[:, 0:1], scale=bnA1[:, 0:1])
                            ups = y1[:].rearrange("c (r w) -> c r w", r=16).unsqueeze(2).unsqueeze(4) \
                                .broadcast_to([128, 16, 2, 16, 2])
                            nc.vector.tensor_copy(x2v[:, t, 1:33, 1:33], ups)

                # ======== Phase 2: interleaved Layer 2 + Layer 3 ========
                with tc.tile_pool(name="p2t", bufs=2) as tp2, \
                     tc.tile_pool(name="ps2", bufs=1, space="PSUM") as pp2, \
                     tc.tile_pool(name="ps3", bufs=2, space="PSUM") as pp3:
                    wx2v = wx2[:].rearrange("c (k m) -> c k m", k=9)
                    wh2pv = wh2p[:].rearrange("c (k m) -> c k m", k=3)
                    wh2sv = wh2s[:].rearrange("c (k m) -> c k m", k=3)
                    h2Av = h2A[:].rearrange("c (r w) -> c r w", r=34)
                    h2Bv = h2B[:].rearrange("c (r w) -> c r w", r=34)
                    y2v = y2sb[:].rearrange("c (r w) -> c r w", r=64)
                    wx3pv = wx3p[:].rearrange("c (k m) -> c k m", k=3)
                    wx3sv = wx3s[:].rearrange("c (k m) -> c k m", k=3)
                    wh3pv = wh3p[:].rearrange("c (k m) -> c k m", k=3)
                    h3Av = h3A[:].rearrange("c (r w) -> c r w", r=34)
                    h3Bv = h3B[:].rearrange("c (r w) -> c r w", r=34)
                    o3v = o3_d[:].rearrange("t c (r w) -> t c r w", r=16)

                    for t in range(T):
                        # ---------- L2 timestep t ----------
                        h2pv = h2Av if t % 2 == 0 else h2Bv   # h(t-1)
                        h2cv = h2Bv if t % 2 == 0 else h2Av   # h(t)
                        GA = pp2.tile([128, 1024], F32, tag="GA")  # f | i
                        GB = pp2.tile([128, 1024], F32, tag="GB")  # o | g
                        for blk, gout in ((0, GA), (1, GB)):
                            ms = slice(blk * 128, blk * 128 + 128)
                            for pc in range(2):
                                out = gout[:, pc * 512:(pc + 1) * 512]
                                r0 = 1 + 16 * pc
                                first = True
                                for tau in range(9):
                                    dy, dx = tau // 3 - 1, tau % 3 - 1
                                    win = x2v[:, t, r0 + dy:r0 + 16 + dy, 1 + dx:33 + dx]
                                    nc.tensor.matmul(out, wx2v[:, tau, ms], win,
                                                     start=first, stop=(t == 0 and tau == 8))
                                    first = False
                                if t == 0:
                                    continue              # h==0 at t=0
                                for dyi in range(3):
                                    dy = dyi - 1
                                    winp = h2pv[:, r0 + dy:r0 + 16 + dy, 1:33]
                                    nc.tensor.matmul(out, wh2pv[:, dyi, ms], winp,
                                                     start=False, stop=False)
                                for dyi in range(3):
                                    dy = dyi - 1
                                    wins = h2pv[:, r0 + dy:r0 + 16 + dy, 3:35]
                                    nc.tensor.matmul(out, wh2sv[:, dyi, ms], wins,
                                                     start=False, stop=(dyi == 2))
                        UA = tp2.tile([128, 1024], F32, tag="UA")
                        Uo2 = tp2.tile([64, 1024], F32, tag="Uo2")
                        P2f = tp2.tile([64, 1024], F32, tag="P2f")
                        P2i = tp2.tile([64, 1024], F32, tag="P2i")
                        tc2 = tp2.tile([64, 1024], F32, tag="tc2")
                        y2f = tp2.tile([64, 1024], BF16, tag="y2f")
                        nc.scalar.activation(S2[64:128, :], GB[64:128, :], AF.Tanh)
                        nc.scalar.activation(UA[:], GA[:], AF.Relu, bias=half[:, 0:1], scale=0.2)
                        nc.scalar.activation(Uo2[:], GB[0:64, :], AF.Relu, bias=half[0:64, 0:1], scale=0.2)
                        if t == 0:                        # c==0: c = i_hat*tg
                            nc.vector.scalar_tensor_tensor(
                                out=S2[0:64, :], in0=UA[64:128, :], scalar=1.0, in1=S2[64:128, :],
                                op0=OP.min, op1=OP.mult)
                        else:
                            nc.vector.scalar_tensor_tensor(
                                out=P2f[:], in0=UA[0:64, :], scalar=1.0, in1=S2[0:64, :],
                                op0=OP.min, op1=OP.mult)
                            nc.vector.scalar_tensor_tensor(
                                out=P2i[:], in0=UA[64:128, :], scalar=1.0, in1=S2[64:128, :],
                                op0=OP.min, op1=OP.mult)
                            nc.vector.tensor_tensor(out=S2[0:64, :], in0=P2f[:], in1=P2i[:], op=OP.add)
                        nc.scalar.activation(tc2[:], S2[0:64, :], AF.Tanh)
                        nc.vector.scalar_tensor_tensor(
                            out=h2cv[0:64, 1:33, 2:34], in0=Uo2[:], scalar=1.0, in1=tc2[:],
                            op0=OP.min, op1=OP.mult)
                        nc.vector.tensor_copy(h2cv[64:128, 1:33, 1:34], h2cv[0:64, 1:33, 2:35])
                        # BN + upsample -> y2sb
                        nc.scalar.activation(y2f[:], h2cv[0:64, 1:33, 2:34],
                                             AF.Identity, bias=bnB2[:, 0:1], scale=bnA2[:, 0:1])
                        ups2 = y2f[:].rearrange("c (r w) -> c r w", r=32).unsqueeze(2).unsqueeze(4) \
                            .broadcast_to([64, 32, 2, 32, 2])
                        nc.vector.tensor_copy(y2v[:, :, 2:66], ups2)
                        nc.scalar.dma_start(st2v[t % 2, :, 9:73, :], y2v[:])

                        # ---------- L3 timestep t ----------
                        a, b_ = 2 + t, 32 - t   # backward-induced need: out rows 9:25 at t=7
                        rows = b_ - a
                        hpv = h3Av if t % 2 == 0 else h3Bv   # h(t-1), read by h-convs
                        hcv = h3Bv if t % 2 == 0 else h3Av   # h(t), written by elementwise
                        x3 = (x3a if t % 2 == 0 else x3b)
                        x3v = x3[:].rearrange("c (r w) -> c r w", r=34)
                        nc.gpsimd.dma_start(x3v[0:64, :, :],
                                            st2v[t % 2, :, bass.ds(sv3, 34), :])
                        nc.gpsimd.dma_start(x3v[64:128, :, 0:67],
                                            st2v[t % 2, :, bass.ds(sv3, 34), 1:68])
                        nblk = (rows + 7) // 8
                        nb0 = (nblk + 1) // 2
                        mid = min(b_, a + 8 * nb0)
                        for (ca, cb) in ((a, mid), (mid, b_)):
                            crows = cb - ca
                            G3c = pp3.tile([128, 1024], F32, tag="G3", name=f"G3_{t}_{ca}")
                            npc = (crows + 7) // 8
                            for pc in range(npc):
                                r0 = ca + 8 * pc
                                r1 = min(cb, r0 + 8)
                                out = G3c[:, (r0 - ca) * 64:(r1 - ca) * 64]
                                first = True
                                for dyi in range(3):
                                    dy = dyi - 1
                                    winp = x3v[:, r0 + dy:r1 + dy, 1:65]
                                    nc.tensor.matmul(out, wx3pv[:, dyi, :], winp,
                                                     start=first, stop=False)
                                    first = False
                                    wins = x3v[:, r0 + dy:r1 + dy, 3:67]
                                    nc.tensor.matmul(out, wx3sv[:, dyi, :], wins,
                                                     start=False, stop=(t == 0 and dyi == 2))
                                if t == 0:
                                    continue              # h==0 at t=0
                                for dyi in range(3):
                                    dy = dyi - 1
                                    winh = hpv[:, r0 + dy:r1 + dy, 1:65]
                                    nc.tensor.matmul(out, wh3pv[:, dyi, :], winh,
                                                     start=False, stop=(dyi == 2))
                            Fc = crows * 64
                            sroi = slice(ca * 64, cb * 64)
                            U3 = tp2.tile([96, 1024], F32, tag="U3")
                            Um = tp2.tile([32, 1024], F32, tag="Um")
                            P3f = tp2.tile([32, 1024], F32, tag="P3f")
                            P3i = tp2.tile([32, 1024], F32, tag="P3i")
                            tc3 = tp2.tile([32, 1024], F32, tag="tc3")
                            nc.scalar.activation(S3[32:64, sroi], G3c[96:128, 0:Fc], AF.Tanh)
                            nc.scalar.activation(U3[:, 0:Fc], G3c[0:96, 0:Fc], AF.Relu,
                                                 bias=half[0:96, 0:1], scale=0.2)
                            mview = rowmask[64:96, ca:cb].unsqueeze(2).broadcast_to([32, crows, 64])
                            nc.vector.tensor_tensor(
                                out=Um[:, 0:Fc].rearrange("c (r w) -> c r w", w=64),
                                in0=U3[64:96, 0:Fc].rearrange("c (r w) -> c r w", w=64),
                                in1=mview, op=OP.mult)
                            if t == 0:                    # c==0: c = i_hat*tg
                                nc.vector.scalar_tensor_tensor(
                                    out=S3[0:32, sroi], in0=U3[32:64, 0:Fc], scalar=1.0,
                                    in1=S3[32:64, sroi], op0=OP.min, op1=OP.mult)
                            else:
                                nc.vector.scalar_tensor_tensor(
                                    out=P3f[:, 0:Fc], in0=U3[0:32, 0:Fc], scalar=1.0,
                                    in1=S3[0:32, sroi], op0=OP.min, op1=OP.mult)
                                nc.vector.scalar_tensor_tensor(
                                    out=P3i[:, 0:Fc], in0=U3[32:64, 0:Fc], scalar=1.0,
                                    in1=S3[32:64, sroi], op0=OP.min, op1=OP.mult)
                                nc.vector.tensor_tensor(out=S3[0:32, sroi], in0=P3f[:, 0:Fc],
                                                        in1=P3i[:, 0:Fc], op=OP.add)
                            nc.scalar.activation(tc3[:, 0:Fc], S3[0:32, sroi], AF.Tanh)
                            nc.vector.tensor_tensor(
                                out=hcv[0:32, ca:cb, 2:66],
                                in0=Um[:, 0:Fc].rearrange("c (r w) -> c r w", w=64),
                                in1=tc3[:, 0:Fc].rearrange("c (r w) -> c r w", w=64), op=OP.mult)
                            nc.vector.tensor_copy(hcv[32:64, ca:cb, 1:65], hcv[0:32, ca:cb, 2:66])
                            nc.vector.tensor_copy(hcv[64:96, ca:cb, 1:65], hcv[0:32, ca:cb, 3:67])
                        nc.sync.dma_start(o3v[t], hcv[0:32, 9:25, 2:66])

    nc.compile()
    return nc


def _prep_inputs(inputs):
    """Build the 8 per-core input maps from the full problem inputs."""
    x = np.asarray(inputs["x"], np.float32)
    W9 = {}
    for l in (1, 2, 3):
        W9[f"x{l}"] = np.asarray(inputs[f"Wx{l}"], np.float32).reshape(9, *inputs[f"Wx{l}"].shape[2:])
        W9[f"h{l}"] = np.asarray(inputs[f"Wh{l}"], np.float32).reshape(9, *inputs[f"Wh{l}"].shape[2:])

    def bn(l, C):
        g = np.asarray(inputs[f"g{l}"], np.float32)
        be = np.asarray(inputs[f"be{l}"], np.float32)
        mm = np.asarray(inputs[f"mm{l}"], np.float32)
        mv = np.asarray(inputs[f"mv{l}"], np.float32)
        A = g / np.sqrt(mv + EPS)
        Bc = be - mm * A
        return A.reshape(C, 1), Bc.reshape(C, 1)

    A1, B1 = bn(1, 128)
    A2, B2 = bn(2, 64)
    A3, B3 = bn(3, 32)

    def padk(a, k=128):
        return np.concatenate([a, np.zeros((k - a.shape[0],) + a.shape[1:], a.dtype)], axis=0)

    wx1 = W9["x1"]  # [9, 192, 512]
    wx1a = wx1[:, 0:128].transpose(1, 0, 2).reshape(128, -1)
    wx1bp = np.concatenate(
        [np.concatenate([wx1[3 * dy + 0, 128:192], wx1[3 * dy + 1, 128:192]], axis=0)[None]
         for dy in range(3)], axis=0).transpose(1, 0, 2).reshape(128, -1)
    wx1bs = padk(wx1[[2, 5, 8], 128:192].transpose(1, 0, 2).reshape(64, -1))
    wh1 = W9["h1"].transpose(1, 0, 2).reshape(128, -1)

    p2 = np.r_[64:128, 0:64, 192:256, 128:192]  # [f,i,o,g]
    wx2 = W9["x2"][:, :, p2].transpose(1, 0, 2).reshape(128, -1)
    wh2 = W9["h2"][:, :, p2]  # [9, 64, 256]
    wh2p = np.concatenate(
        [np.concatenate([wh2[3 * dy + 0], wh2[3 * dy + 1]], axis=0)[None]
         for dy in range(3)], axis=0).transpose(1, 0, 2).reshape(128, -1)
    wh2s = padk(wh2[[2, 5, 8]].transpose(1, 0, 2).reshape(64, -1))

    p3 = np.r_[32:64, 0:32, 96:128, 64:96]  # [f,i,o,g]
    wx3 = W9["x3"][:, :, p3]  # [9, 64, 128]
    wx3p = np.concatenate(
        [np.concatenate([wx3[3 * dy + 0], wx3[3 * dy + 1]], axis=0)[None]
         for dy in range(3)], axis=0).transpose(1, 0, 2).reshape(128, -1)
    wx3s = padk(wx3[[2, 5, 8]].transpose(1, 0, 2).reshape(64, -1))
    wh3 = W9["h3"][:, :, p3]  # [9, 32, 128]
    wh3p = padk(np.concatenate(
        [np.concatenate([wh3[3 * dy + 0], wh3[3 * dy + 1], wh3[3 * dy + 2]], axis=0)[None]
         for dy in range(3)], axis=0).transpose(1, 0, 2).reshape(96, -1))

    shared = dict(wx1a=wx1a, wx1bp=wx1bp, wx1bs=wx1bs, wh1=wh1,
                  wx2=wx2, wh2p=wh2p, wh2s=wh2s,
                  wx3p=wx3p, wx3s=wx3s, wh3p=wh3p,
                  bnA1=A1, bnB1=B1, bnA2=A2, bnB2=B2)
    import ml_dtypes
    bf = ml_dtypes.bfloat16
    bf_keys = {"wx1a", "wx1bp", "wx1bs", "wh1", "wx2", "wh2p", "wh2s",
               "wx3p", "wx3s", "wh3p"}
    shared = {k: (np.ascontiguousarray(v).astype(bf) if k in bf_keys
                  else np.ascontiguousarray(v, np.float32))
              for k, v in shared.items()}

    maps = []
    for core in range(8):
        b = core // 4
        q = core % 4
        xp = np.zeros((T, 192, 18, 18), np.float32)
        xp[:, :, 1:17, 1:17] = x[b].transpose(0, 3, 1, 2)
        x1a = np.ascontiguousarray(xp[:, 0:128].transpose(1, 0, 2, 3).reshape(128, -1))
        xb = xp[:, 128:192]
        xbs = np.zeros_like(xb)
        xbs[..., :-1] = xb[..., 1:]
        x1b = np.ascontiguousarray(
            np.concatenate([xb, xbs], axis=1).transpose(1, 0, 2, 3).reshape(128, -1))
        m = dict(shared)
        m["x1a"] = x1a.astype(bf)
        m["x1b"] = x1b.astype(bf)
        m["shard_off"] = np.array([[16 * q]], np.int32)
        gl = 16 * q - 9 + np.arange(34)
        mask = ((gl >= 0) & (gl < 64)).astype(np.float32)
        m["rowmask"] = np.ascontiguousarray(np.broadcast_to(mask, (96, 34)))
        maps.append(m)
    return maps, (A3.reshape(32), B3.reshape(32))


def kernel(**inputs):
    global _PROG
    if _PROG is None:
        _PROG = build_program()
    nc = _PROG
    maps, (A3, B3) = _prep_inputs(inputs)

    trace = bool(os.environ.get("KERNEL_TRACE"))
    kw = {}
    if trace and _install_trace_hook():
        kw = dict(trace=True, trace_cores=[0])
    res = run_bass_kernel_spmd(nc, maps, core_ids=list(range(8)), **kw)
    if trace:
        kernel.last_exec_ns = res.exec_time_ns

    # assemble: core b*4+q provides H3 rows [16q, 16q+16)
    h3 = np.zeros((B, T, 32, 64, 64), np.float32)
    for core in range(8):
        b, q = core // 4, core % 4
        h3[b, :, :, 16 * q:16 * q + 16, :] = \
            res.results[core]["o3"].astype(np.float32).reshape(T, 32, 16, 64)
    y = h3 * A3[None, None, :, None, None] + B3[None, None, :, None, None]
    y = np.repeat(np.repeat(y, 2, axis=3), 2, axis=4)  # [B,T,32,128,128]
    return np.ascontiguousarray(y.transpose(0, 1, 3, 4, 2))


kernel.last_exec_ns = None


# revision 59
# speedup vs baseline: 1.0259x; 1.0242x over previous
"""Trainium2 Bass kernel for nn_DecCLSTMBlock (3x stacked ConvLSTM2D + BN + 2x2 upsample).

Sharding: 8 cores = 2 batch groups x 4 H-shards.
  - L1 (16x16, Cout=128) and L2 (32x32, Cout=64) computed fully per core
    (replicated within the batch group).
  - L3 (64x64, Cout=32) sharded 4 ways over H with a shrinking-halo
    overcompute domain of 34 rows -> no inter-core communication.
  - L2 and L3 are interleaved per timestep: L3's elementwise-heavy chain
    overlaps L2's matmul-heavy stream. y2 stays in SBUF (no DRAM staging).
  - Final BN + upsample of layer 3 done host-side in numpy.
All conv weights are zero-padded to K=128 partitions so every LDWEIGHTS has
identical full-array geometry (lets the PE pull weight loads ahead of
in-flight matmuls). LSTM cell elementwise uses scalar_tensor_tensor fusion:
(U min 1) * v does hard_sigmoid clamp + gate multiply in one op.
Matmuls run in bf16 with fp32 PSUM accumulation (cell state c stays fp32).
Assumes bias vectors b1..b3 are zero (they are, per the problem spec).
"""
import os
import sys
import types

sys.path.insert(0, "/opt/trn_rl_repo")

import numpy as np

import concourse.bass as bass
import concourse.tile as tile
from concourse import bacc, mybir
from concourse.bass_utils import run_bass_kernel_spmd

F32 = mybir.dt.float32
BF16 = mybir.dt.bfloat16
I32 = mybir.dt.int32
AF = mybir.ActivationFunctionType
OP = mybir.AluOpType

B, T = 2, 8
EPS = 1e-3

_PROG = None


def _install_trace_hook():
    try:
        import antenv
        if 'antenv.axon_hooks' not in sys.modules:
            mod = types.ModuleType('antenv.axon_hooks')
            mod._hook = None
            def _set(h):
                mod._hook = h
            def _get():
                return mod._hook
            mod.set_axon_ntff_profile_hook = _set
            mod.get_axon_ntff_profile_hook = _get
            sys.modules['antenv.axon_hooks'] = mod
            antenv.axon_hooks = mod
            from trn_agent_boot.trn_boot import _ntff_profile_via_ctypes
            mod.set_axon_ntff_profile_hook(
                _ntff_profile_via_ctypes('/opt/axon/libaxon_pjrt.so'))
        import concourse.bass_utils as bu
        bu.upload_artifacts = lambda tmpdir: "local://" + tmpdir
        return True
    except Exception:
        return False


def build_program():
    nc = bacc.Bacc("TRN2", target_bir_lowering=False, debug=False, num_devices=8)
    dt_in = {}

    def din(name, shape, dt=F32):
        dt_in[name] = nc.dram_tensor(name, list(shape), dt, kind="ExternalInput")
        return dt_in[name]

    x1a_d = din("x1a", [128, T * 18 * 18], BF16)
    x1b_d = din("x1b", [128, T * 18 * 18], BF16)
    wx1a_d = din("wx1a", [128, 9 * 512], BF16)
    wx1bp_d = din("wx1bp", [128, 3 * 512], BF16)
    wx1bs_d = din("wx1bs", [128, 3 * 512], BF16)
    wh1_d = din("wh1", [128, 9 * 512], BF16)
    wx2_d = din("wx2", [128, 9 * 256], BF16)
    wh2p_d = din("wh2p", [128, 3 * 256], BF16)
    wh2s_d = din("wh2s", [128, 3 * 256], BF16)
    wx3p_d = din("wx3p", [128, 3 * 128], BF16)
    wx3s_d = din("wx3s", [128, 3 * 128], BF16)
    wh3p_d = din("wh3p", [128, 3 * 128], BF16)
    bnA1_d = din("bnA1", [128, 1])
    bnB1_d = din("bnB1", [128, 1])
    bnA2_d = din("bnA2", [64, 1])
    bnB2_d = din("bnB2", [64, 1])
    off_d = din("shard_off", [1, 1], I32)
    msk_d = din("rowmask", [96, 34])

    o3_d = nc.dram_tensor("o3", [T, 32, 16 * 64], BF16, kind="ExternalOutput")

    with tile.TileContext(nc) as tc:
        with tc.tile_pool(name="glob", bufs=1) as gp, \
             tc.tile_pool(name="dram", bufs=1, space="DRAM") as dp:
            stage2 = dp.tile([2 * 64, 82 * 68], BF16)   # 2-slot y2 ring
            # ---- persistent state ----
            h3A = gp.tile([128, 34 * 68], BF16)     # h | rep+1 | rep+2 | zeros
            h3B = gp.tile([128, 34 * 68], BF16)     # ping-pong partner of h3A
            S3 = gp.tile([64, 34 * 64], F32)        # c | tg (abs rows)
            wx3p = gp.tile([128, 3 * 128], BF16)
            wx3s = gp.tile([128, 3 * 128], BF16)
            wh3p = gp.tile([128, 3 * 128], BF16)
            offt = gp.tile([1, 1], I32)
            rowmask = gp.tile([96, 34], F32)
            half = gp.tile([128, 1], F32)
            x3a = gp.tile([128, 34 * 68], BF16)
            x3b = gp.tile([128, 34 * 68], BF16)
            y2sb = gp.tile([64, 64 * 68], BF16)     # one timestep of padded y2

            # spread initial loads across DMA queues (sync gets L1-critical)
            nc.gpsimd.dma_start(wx3p[:], wx3p_d[:])
            nc.gpsimd.dma_start(wx3s[:], wx3s_d[:])
            nc.gpsimd.dma_start(wh3p[:], wh3p_d[:])
            nc.gpsimd.dma_start(offt[:], off_d[:])
            nc.gpsimd.dma_start(rowmask[:], msk_d[:])
            nc.vector.memset(half[:], 0.5)
            nc.vector.memset(h3A[:], 0.0)
            nc.vector.memset(h3B[:], 0.0)
            nc.gpsimd.memset(S3[:], 0.0)
            nc.gpsimd.memset(x3a[:], 0.0)
            nc.gpsimd.memset(x3b[:], 0.0)
            nc.vector.memset(y2sb[:], 0.0)

            rv3 = nc.gpsimd.alloc_register("shardoff3")
            nc.gpsimd.reg_load(rv3, offt[0:1, 0:1])
            sv3 = nc.gpsimd.snap(rv3, donate=True, min_val=0, max_val=48)

            # zero the guard rows of both y2 stage slots (rows 0:9, 73:82)
            st2v = stage2[:].rearrange("(s c) (r w) -> s c r w", s=2, r=82)
            with tc.tile_pool(name="zp", bufs=1) as zp:
                zsrc = zp.tile([64, 9 * 68], BF16)
                nc.vector.memset(zsrc[:], 0.0)
                zv = zsrc[:].rearrange("c (r w) -> c r w", r=9)
                for s in range(2):
                    nc.scalar.dma_start(st2v[s, :, 0:9, :], zv[:])
                    nc.scalar.dma_start(st2v[s, :, 73:82, :], zv[:])

            with tc.tile_pool(name="p12", bufs=1) as p12:
                x2 = p12.tile([128, T * 34 * 34], BF16)
                bnA2 = p12.tile([64, 1], F32)
                bnB2 = p12.tile([64, 1], F32)
                # phase-2 persistent state, allocated before L1's tiles so the
                # loads overlap L1 compute instead of waiting on SBUF reuse
                wx2 = p12.tile([128, 9 * 256], BF16)
                wh2p = p12.tile([128, 3 * 256], BF16)
                wh2s = p12.tile([128, 3 * 256], BF16)
                h2A = p12.tile([128, 34 * 36], BF16)
                h2B = p12.tile([128, 34 * 36], BF16)
                S2 = p12.tile([128, 1024], F32)     # c | tg
                nc.gpsimd.dma_start(bnA2[:], bnA2_d[:])
                nc.gpsimd.dma_start(bnB2[:], bnB2_d[:])
                nc.gpsimd.dma_start(wx2[:], wx2_d[:])
                nc.gpsimd.dma_start(wh2p[:], wh2p_d[:])
                nc.gpsimd.dma_start(wh2s[:], wh2s_d[:])
                nc.gpsimd.memset(x2[:], 0.0)
                nc.gpsimd.memset(h2A[:], 0.0)
                nc.gpsimd.memset(h2B[:], 0.0)
                nc.gpsimd.memset(S2[:], 0.0)
                x2v = x2[:].rearrange("c (t r w) -> c t r w", t=T, r=34)

                # ================= Layer 1 =================
                with tc.tile_pool(name="l1w", bufs=1) as wp, \
                     tc.tile_pool(name="l1t", bufs=2) as tp, \
                     tc.tile_pool(name="ps1", bufs=2, space="PSUM") as pp:
                    x1a = wp.tile([128, T * 18 * 18], BF16)
                    x1b = wp.tile([128, T * 18 * 18], BF16)
                    wx1a = wp.tile([128, 9 * 512], BF16)
                    wx1bp = wp.tile([128, 3 * 512], BF16)
                    wx1bs = wp.tile([128, 3 * 512], BF16)
                    wh1 = wp.tile([128, 9 * 512], BF16)
                    bnA1 = wp.tile([128, 1], F32)
                    bnB1 = wp.tile([128, 1], F32)
                    h1 = wp.tile([128, 18 * 18], BF16)
                    c1 = wp.tile([128, 256], F32)
                    # L1-critical tensors on the sync queue, rest on scalar
                    nc.sync.dma_start(x1a[:], x1a_d[:])
                    nc.sync.dma_start(wx1a[:], wx1a_d[:])
                    nc.scalar.dma_start(x1b[:], x1b_d[:])
                    nc.scalar.dma_start(wx1bp[:], wx1bp_d[:])
                    nc.scalar.dma_start(wx1bs[:], wx1bs_d[:])
                    nc.scalar.dma_start(wh1[:], wh1_d[:])
                    nc.scalar.dma_start(bnA1[:], bnA1_d[:])
                    nc.scalar.dma_start(bnB1[:], bnB1_d[:])
                    nc.vector.memset(h1[:], 0.0)
                    nc.vector.memset(c1[:], 0.0)

                    x1av = x1a[:].rearrange("c (t r w) -> c t r w", t=T, r=18)
                    x1bv = x1b[:].rearrange("c (t r w) -> c t r w", t=T, r=18)
                    wx1av = wx1a[:].rearrange("c (k m) -> c k m", k=9)
                    wx1bpv = wx1bp[:].rearrange("c (k m) -> c k m", k=3)
                    wx1bsv = wx1bs[:].rearrange("c (k m) -> c k m", k=3)
                    wh1v = wh1[:].rearrange("c (k m) -> c k m", k=9)
                    h1v = h1[:].rearrange("c (r w) -> c r w", r=18)

                    for tpair in range(T // 2):
                        t0 = 2 * tpair
                        Gg = [pp.tile([128, 512], F32, tag=f"G{g}", name=f"G{g}_{tpair}")
                              for g in range(4)]
                        for g in range(4):
                            ms = slice(g * 128, g * 128 + 128)
                            first = True
                            for tau in range(9):
                                dy, dx = tau // 3 - 1, tau % 3 - 1
                                win = x1av[:, t0:t0 + 2, 1 + dy:17 + dy, 1 + dx:17 + dx]
                                nc.tensor.matmul(Gg[g][:], wx1av[:, tau, ms], win,
                                                 start=first, stop=False)
                                first = False
                            for dyi in range(3):
                                dy = dyi - 1
                                winp = x1bv[:, t0:t0 + 2, 1 + dy:17 + dy, 0:16]
                                nc.tensor.matmul(Gg[g][:], wx1bpv[:, dyi, ms], winp,
                                                 start=False, stop=False)
                            for dyi in range(3):
                                dy = dyi - 1
                                wins = x1bv[:, t0:t0 + 2, 1 + dy:17 + dy, 2:18]
                                nc.tensor.matmul(Gg[g][:], wx1bsv[:, dyi, ms], wins,
                                                 start=False, stop=False)
                        for p in range(2):
                            t = t0 + p
                            col = slice(p * 256, p * 256 + 256)
                            if not (tpair == 0 and p == 0):   # h==0 at t=0
                                for g in (2, 0, 1, 3):        # g-gate first: tanh heads the chain
                                    ms = slice(g * 128, g * 128 + 128)
                                    for tau in range(9):
                                        dy, dx = tau // 3 - 1, tau % 3 - 1
                                        win = h1v[:, 1 + dy:17 + dy, 1 + dx:17 + dx]
                                        nc.tensor.matmul(Gg[g][:, col], wh1v[:, tau, ms], win,
                                                         start=False, stop=(p == 1 and tau == 8))
                            # elementwise: gate order i,f,g,o in Gg[0..3]
                            tg1 = tp.tile([128, 256], F32, tag="tg1")
                            Uf = tp.tile([128, 256], F32, tag="Uf")
                            Ui = tp.tile([128, 256], F32, tag="Ui")
                            Uo = tp.tile([128, 256], F32, tag="Uo")
                            Pf = tp.tile([128, 256], F32, tag="Pf")
                            Pi = tp.tile([128, 256], F32, tag="Pi")
                            tc1 = tp.tile([128, 256], F32, tag="tc1")
                            y1 = tp.tile([128, 256], BF16, tag="y1")
                            nc.scalar.activation(tg1[:], Gg[2][:, col], AF.Tanh)
                            nc.scalar.activation(Ui[:], Gg[0][:, col], AF.Relu, bias=half[:, 0:1], scale=0.2)
                            nc.scalar.activation(Uo[:], Gg[3][:, col], AF.Relu, bias=half[:, 0:1], scale=0.2)
                            if tpair == 0 and p == 0:         # c==0: c1 = i_hat*tg
                                nc.vector.scalar_tensor_tensor(
                                    out=c1[:], in0=Ui[:], scalar=1.0, in1=tg1[:],
                                    op0=OP.min, op1=OP.mult)
                            else:
                                nc.scalar.activation(Uf[:], Gg[1][:, col], AF.Relu, bias=half[:, 0:1], scale=0.2)
                                nc.vector.scalar_tensor_tensor(
                                    out=Pf[:], in0=Uf[:], scalar=1.0, in1=c1[:],
                                    op0=OP.min, op1=OP.mult)
                                nc.vector.scalar_tensor_tensor(
                                    out=Pi[:], in0=Ui[:], scalar=1.0, in1=tg1[:],
                                    op0=OP.min, op1=OP.mult)
                                nc.vector.tensor_tensor(out=c1[:], in0=Pf[:], in1=Pi[:], op=OP.add)
                            nc.scalar.activation(tc1[:], c1[:], AF.Tanh)
                            nc.vector.scalar_tensor_tensor(
                                out=h1v[:, 1:17, 1:17], in0=Uo[:], scalar=1.0, in1=tc1[:],
                                op0=OP.min, op1=OP.mult)
                            # BN + 2x upsample -> x2[t] interior
                            nc.scalar.activation(y1[:], h1v[:, 1:17, 1:17],
                                                 AF.Identity, bias=bnB1[:, 0:1], scale=bnA1[:, 0:1])
                            ups = y1[:].rearrange("c (r w) -> c r w", r=16).unsqueeze(2).unsqueeze(4) \
                                .broadcast_to([128, 16, 2, 16, 2])
                            nc.vector.tensor_copy(x2v[:, t, 1:33, 1:33], ups)

                # ======== Phase 2: interleaved Layer 2 + Layer 3 ========
                with tc.tile_pool(name="p2t", bufs=2) as tp2, \
                     tc.tile_pool(name="ps2", bufs=1, space="PSUM") as pp2, \
                     tc.tile_pool(name="ps3", bufs=2, space="PSUM") as pp3:
                    wx2v = wx2[:].rearrange("c (k m) -> c k m", k=9)
                    wh2pv = wh2p[:].rearrange("c (k m) -> c k m", k=3)
                    wh2sv = wh2s[:].rearrange("c (k m) -> c k m", k=3)
                    h2Av = h2A[:].rearrange("c (r w) -> c r w", r=34)
                    h2Bv = h2B[:].rearrange("c (r w) -> c r w", r=34)
                    y2v = y2sb[:].rearrange("c (r w) -> c r w", r=64)
                    wx3pv = wx3p[:].rearrange("c (k m) -> c k m", k=3)
                    wx3sv = wx3s[:].rearrange("c (k m) -> c k m", k=3)
                    wh3pv = wh3p[:].rearrange("c (k m) -> c k m", k=3)
                    h3Av = h3A[:].rearrange("c (r w) -> c r w", r=34)
                    h3Bv = h3B[:].rearrange("c (r w) -> c r w", r=34)
                    o3v = o3_d[:].rearrange("t c (r w) -> t c r w", r=16)

                    for t in range(T):
                        # ---------- L2 timestep t ----------
                        h2pv = h2Av if t % 2 == 0 else h2Bv   # h(t-1)
                        h2cv = h2Bv if t % 2 == 0 else h2Av   # h(t)
                        GA = pp2.tile([128, 1024], F32, tag="GA")  # f | i
                        GB = pp2.tile([128, 1024], F32, tag="GB")  # o | g
                        for blk, gout in ((0, GA), (1, GB)):
                            ms = slice(blk * 128, blk * 128 + 128)
                            for pc in range(2):
                                out = gout[:, pc * 512:(pc + 1) * 512]
                                r0 = 1 + 16 * pc
                                first = True
                                for tau in range(9):
                                    dy, dx = tau // 3 - 1, tau % 3 - 1
                                    win = x2v[:, t, r0 + dy:r0 + 16 + dy, 1 + dx:33 + dx]
                                    nc.tensor.matmul(out, wx2v[:, tau, ms], win,
                                                     start=first, stop=(t == 0 and tau == 8))
                                    first = False
                                if t == 0:
                                    continue              # h==0 at t=0
                                for dyi in range(3):
                                    dy = dyi - 1
                                    winp = h2pv[:, r0 + dy:r0 + 16 + dy, 1:33]
                                    nc.tensor.matmul(out, wh2pv[:, dyi, ms], winp,
                                                     start=False, stop=False)
                                for dyi in range(3):
                                    dy = dyi - 1
                                    wins = h2pv[:, r0 + dy:r0 + 16 + dy, 3:35]
                                    nc.tensor.matmul(out, wh2sv[:, dyi, ms], wins,
                                                     start=False, stop=(dyi == 2))
                        UA = tp2.tile([128, 1024], F32, tag="UA")
                        Uo2 = tp2.tile([64, 1024], F32, tag="Uo2")
                        P2f = tp2.tile([64, 1024], F32, tag="P2f")
                        P2i = tp2.tile([64, 1024], F32, tag="P2i")
                        tc2 = tp2.tile([64, 1024], F32, tag="tc2")
                        y2f = tp2.tile([64, 1024], BF16, tag="y2f")
                        nc.scalar.activation(S2[64:128, :], GB[64:128, :], AF.Tanh)
                        nc.scalar.activation(UA[:], GA[:], AF.Relu, bias=half[:, 0:1], scale=0.2)
                        nc.scalar.activation(Uo2[:], GB[0:64, :], AF.Relu, bias=half[0:64, 0:1], scale=0.2)
                        if t == 0:                        # c==0: c = i_hat*tg
                            nc.vector.scalar_tensor_tensor(
                                out=S2[0:64, :], in0=UA[64:128, :], scalar=1.0, in1=S2[64:128, :],
                                op0=OP.min, op1=OP.mult)
                        else:
                            nc.vector.scalar_tensor_tensor(
                                out=P2f[:], in0=UA[0:64, :], scalar=1.0, in1=S2[0:64, :],
                                op0=OP.min, op1=OP.mult)
                            nc.vector.scalar_tensor_tensor(
                                out=P2i[:], in0=UA[64:128, :], scalar=1.0, in1=S2[64:128, :],
                                op0=OP.min, op1=OP.mult)
                            nc.vector.tensor_tensor(out=S2[0:64, :], in0=P2f[:], in1=P2i[:], op=OP.add)
                        nc.scalar.activation(tc2[:], S2[0:64, :], AF.Tanh)
                        nc.vector.scalar_tensor_tensor(
                            out=h2cv[0:64, 1:33, 2:34], in0=Uo2[:], scalar=1.0, in1=tc2[:],
                            op0=OP.min, op1=OP.mult)
                        nc.vector.tensor_copy(h2cv[64:128, 1:33, 1:34], h2cv[0:64, 1:33, 2:35])
                        # BN + upsample -> y2sb
                        nc.scalar.activation(y2f[:], h2cv[0:64, 1:33, 2:34],
                                             AF.Identity, bias=bnB2[:, 0:1], scale=bnA2[:, 0:1])
                        ups2 = y2f[:].rearrange("c (r w) -> c r w", r=32).unsqueeze(2).unsqueeze(4) \
                            .broadcast_to([64, 32, 2, 32, 2])
                        nc.vector.tensor_copy(y2v[:, :, 2:66], ups2)
                        nc.scalar.dma_start(st2v[t % 2, :, 9:73, :], y2v[:])

                        # ---------- L3 timestep t ----------
                        a, b_ = 2 + t, 32 - t   # backward-induced need: out rows 9:25 at t=7
                        rows = b_ - a
                        hpv = h3Av if t % 2 == 0 else h3Bv   # h(t-1), read by h-convs
                        hcv = h3Bv if t % 2 == 0 else h3Av   # h(t), written by elementwise
                        x3 = (x3a if t % 2 == 0 else x3b)
                        x3v = x3[:].rearrange("c (r w) -> c r w", r=34)
                        nc.gpsimd.dma_start(x3v[0:64, :, :],
                                            st2v[t % 2, :, bass.ds(sv3, 34), :])
                        nc.gpsimd.dma_start(x3v[64:128, :, 0:67],
                                            st2v[t % 2, :, bass.ds(sv3, 34), 1:68])
                        nblk = (rows + 7) // 8
                        if rows <= 16:
                            chunks = [(a, b_)]          # one chunk fits a G3 tile
                        else:
                            mid = a + 8 * (nblk // 2)   # balanced split
                            chunks = [(a, mid), (mid, b_)]
                        for (ca, cb) in chunks:
                            crows = cb - ca
                            G3c = pp3.tile([128, 1024], F32, tag="G3", name=f"G3_{t}_{ca}")
                            npc = (crows + 7) // 8
                            for pc in range(npc):
                                r0 = ca + 8 * pc
                                r1 = min(cb, r0 + 8)
                                out = G3c[:, (r0 - ca) * 64:(r1 - ca) * 64]
                                first = True
                                for dyi in range(3):
                                    dy = dyi - 1
                                    winp = x3v[:, r0 + dy:r1 + dy, 1:65]
                                    nc.tensor.matmul(out, wx3pv[:, dyi, :], winp,
                                                     start=first, stop=False)
                                    first = False
                                    wins = x3v[:, r0 + dy:r1 + dy, 3:67]
                                    nc.tensor.matmul(out, wx3sv[:, dyi, :], wins,
                                                     start=False, stop=(t == 0 and dyi == 2))
                                if t == 0:
                                    continue              # h==0 at t=0
                                for dyi in range(3):
                                    dy = dyi - 1
                                    winh = hpv[:, r0 + dy:r1 + dy, 1:65]
                                    nc.tensor.matmul(out, wh3pv[:, dyi, :], winh,
                                                     start=False, stop=(dyi == 2))
                            Fc = crows * 64
                            sroi = slice(ca * 64, cb * 64)
                            U3 = tp2.tile([96, 1024], F32, tag="U3")
                            Um = tp2.tile([32, 1024], F32, tag="Um")
                            P3f = tp2.tile([32, 1024], F32, tag="P3f")
                            P3i = tp2.tile([32, 1024], F32, tag="P3i")
                            tc3 = tp2.tile([32, 1024], F32, tag="tc3")
                            nc.scalar.activation(S3[32:64, sroi], G3c[96:128, 0:Fc], AF.Tanh)
                            nc.scalar.activation(U3[:, 0:Fc], G3c[0:96, 0:Fc], AF.Relu,
                                                 bias=half[0:96, 0:1], scale=0.2)
                            mview = rowmask[64:96, ca:cb].unsqueeze(2).broadcast_to([32, crows, 64])
                            nc.vector.tensor_tensor(
                                out=Um[:, 0:Fc].rearrange("c (r w) -> c r w", w=64),
                                in0=U3[64:96, 0:Fc].rearrange("c (r w) -> c r w", w=64),
                                in1=mview, op=OP.mult)
                            if t == 0:                    # c==0: c = i_hat*tg
                                nc.vector.scalar_tensor_tensor(
                                    out=S3[0:32, sroi], in0=U3[32:64, 0:Fc], scalar=1.0,
                                    in1=S3[32:64, sroi], op0=OP.min, op1=OP.mult)
                            else:
                                nc.vector.scalar_tensor_tensor(
                                    out=P3f[:, 0:Fc], in0=U3[0:32, 0:Fc], scalar=1.0,
                                    in1=S3[0:32, sroi], op0=OP.min, op1=OP.mult)
                                nc.vector.scalar_tensor_tensor(
                                    out=P3i[:, 0:Fc], in0=U3[32:64, 0:Fc], scalar=1.0,
                                    in1=S3[32:64, sroi], op0=OP.min, op1=OP.mult)
                                nc.vector.tensor_tensor(out=S3[0:32, sroi], in0=P3f[:, 0:Fc],
                                                        in1=P3i[:, 0:Fc], op=OP.add)
                            nc.scalar.activation(tc3[:, 0:Fc], S3[0:32, sroi], AF.Tanh)
                            nc.vector.tensor_tensor(
                                out=hcv[0:32, ca:cb, 2:66],
                                in0=Um[:, 0:Fc].rearrange("c (r w) -> c r w", w=64),
                                in1=tc3[:, 0:Fc].rearrange("c (r w) -> c r w", w=64), op=OP.mult)
                            if t < T - 1:   # replicas only feed the next step's h-convs
                                nc.vector.tensor_copy(hcv[32:64, ca:cb, 1:65], hcv[0:32, ca:cb, 2:66])
                                nc.vector.tensor_copy(hcv[64:96, ca:cb, 1:65], hcv[0:32, ca:cb, 3:67])
                        nc.sync.dma_start(o3v[t], hcv[0:32, 9:25, 2:66])

    nc.compile()
    return nc


def _prep_inputs(inputs):
    """Build the 8 per-core input maps from the full problem inputs."""
    x = np.asarray(inputs["x"], np.float32)
    W9 = {}
    for l in (1, 2, 3):
        W9[f"x{l}"] = np.asarray(inputs[f"Wx{l}"], np.float32).reshape(9, *inputs[f"Wx{l}"].shape[2:])
        W9[f"h{l}"] = np.asarray(inputs[f"Wh{l}"], np.float32).reshape(9, *inputs[f"Wh{l}"].shape[2:])

    def bn(l, C):
        g = np.asarray(inputs[f"g{l}"], np.float32)
        be = np.asarray(inputs[f"be{l}"], np.float32)
        mm = np.asarray(inputs[f"mm{l}"], np.float32)
        mv = np.asarray(inputs[f"mv{l}"], np.float32)
        A = g / np.sqrt(mv + EPS)
        Bc = be - mm * A
        return A.reshape(C, 1), Bc.reshape(C, 1)

    A1, B1 = bn(1, 128)
    A2, B2 = bn(2, 64)
    A3, B3 = bn(3, 32)

    def padk(a, k=128):
        return np.concatenate([a, np.zeros((k - a.shape[0],) + a.shape[1:], a.dtype)], axis=0)

    wx1 = W9["x1"]  # [9, 192, 512]
    wx1a = wx1[:, 0:128].transpose(1, 0, 2).reshape(128, -1)
    wx1bp = np.concatenate(
        [np.concatenate([wx1[3 * dy + 0, 128:192], wx1[3 * dy + 1, 128:192]], axis=0)[None]
         for dy in range(3)], axis=0).transpose(1, 0, 2).reshape(128, -1)
    wx1bs = padk(wx1[[2, 5, 8], 128:192].transpose(1, 0, 2).reshape(64, -1))
    wh1 = W9["h1"].transpose(1, 0, 2).reshape(128, -1)

    p2 = np.r_[64:128, 0:64, 192:256, 128:192]  # [f,i,o,g]
    wx2 = W9["x2"][:, :, p2].transpose(1, 0, 2).reshape(128, -1)
    wh2 = W9["h2"][:, :, p2]  # [9, 64, 256]
    wh2p = np.concatenate(
        [np.concatenate([wh2[3 * dy + 0], wh2[3 * dy + 1]], axis=0)[None]
         for dy in range(3)], axis=0).transpose(1, 0, 2).reshape(128, -1)
    wh2s = padk(wh2[[2, 5, 8]].transpose(1, 0, 2).reshape(64, -1))

    p3 = np.r_[32:64, 0:32, 96:128, 64:96]  # [f,i,o,g]
    wx3 = W9["x3"][:, :, p3]  # [9, 64, 128]
    wx3p = np.concatenate(
        [np.concatenate([wx3[3 * dy + 0], wx3[3 * dy + 1]], axis=0)[None]
         for dy in range(3)], axis=0).transpose(1, 0, 2).reshape(128, -1)
    wx3s = padk(wx3[[2, 5, 8]].transpose(1, 0, 2).reshape(64, -1))
    wh3 = W9["h3"][:, :, p3]  # [9, 32, 128]
    wh3p = padk(np.concatenate(
        [np.concatenate([wh3[3 * dy + 0], wh3[3 * dy + 1], wh3[3 * dy + 2]], axis=0)[None]
         for dy in range(3)], axis=0).transpose(1, 0, 2).reshape(96, -1))

    shared = dict(wx1a=wx1a, wx1bp=wx1bp, wx1bs=wx1bs, wh1=wh1,
                  wx2=wx2, wh2p=wh2p, wh2s=wh2s,
                  wx3p=wx3p, wx3s=wx3s, wh3p=wh3p,
                  bnA1=A1, bnB1=B1, bnA2=A2, bnB2=B2)
    import ml_dtypes
    bf = ml_dtypes.bfloat16
    bf_keys = {"wx1a", "wx1bp", "wx1bs", "wh1", "wx2", "wh2p", "wh2s",
               "wx3p", "wx3s", "wh3p"}
    shared = {k: (np.ascontiguousarray(v).astype(bf) if k in bf_keys
                  else np.ascontiguousarray(v, np.float32))
              for k, v in shared.items()}

    maps = []
    for core in range(8):
        b = core // 4
        q = core % 4
        xp = np.zeros((T, 192, 18, 18), np.float32)
        xp[:, :, 1:17, 1:17] = x[b].transpose(0, 3, 1, 2)
        x1a = np.ascontiguousarray(xp[:, 0:128].transpose(1, 0, 2, 3).reshape(128, -1))
        xb = xp[:, 128:192]
        xbs = np.zeros_like(xb)
        xbs[..., :-1] = xb[..., 1:]
        x1b = np.ascontiguousarray(
            np.concatenate([xb, xbs], axis=1).transpose(1, 0, 2, 3).reshape(128, -1))
        m = dict(shared)
        m["x1a"] = x1a.astype(bf)
        m["x1b"] = x1b.astype(bf)
        m["shard_off"] = np.array([[16 * q]], np.int32)
        gl = 16 * q - 9 + np.arange(34)
        mask = ((gl >= 0) & (gl < 64)).astype(np.float32)
        m["rowmask"] = np.ascontiguousarray(np.broadcast_to(mask, (96, 34)))
        maps.append(m)
    return maps, (A3.reshape(32), B3.reshape(32))


def kernel(**inputs):
    global _PROG
    if _PROG is None:
        _PROG = build_program()
    nc = _PROG
    maps, (A3, B3) = _prep_inputs(inputs)

    trace = bool(os.environ.get("KERNEL_TRACE"))
    kw = {}
    if trace and _install_trace_hook():
        kw = dict(trace=True, trace_cores=[0])
    res = run_bass_kernel_spmd(nc, maps, core_ids=list(range(8)), **kw)
    if trace:
        kernel.last_exec_ns = res.exec_time_ns

    # assemble: core b*4+q provides H3 rows [16q, 16q+16)
    h3 = np.zeros((B, T, 32, 64, 64), np.float32)
    for core in range(8):
        b, q = core // 4, core % 4
        h3[b, :, :, 16 * q:16 * q + 16, :] = \
            res.results[core]["o3"].astype(np.float32).reshape(T, 32, 16, 64)
    y = h3 * A3[None, None, :, None, None] + B3[None, None, :, None, None]
    y = np.repeat(np.repeat(y, 2, axis=3), 2, axis=4)  # [B,T,32,128,128]
    return np.ascontiguousarray(y.transpose(0, 1, 3, 4, 2))


kernel.last_exec_ns = None
